# revision 4
# baseline (speedup 1.0000x reference)
"""CosFormer layer kernel for 8x Trainium2 (Bass/Tile), data-parallel over batch.

v2: fp8e4m3 DoubleRow matmuls for QKV/O/FFN (2x PE throughput), linearized
softmax (exp(s) ~= 1+s and den ~= S: |logits| <= 1/8 makes both exact to
~1e-4 of the output), single ACT table (ln/exp/copy/relu/square only, so no
ACT_TABLE_LOADs), and a Vsum-augmented AV so the attention uniform part
costs one extra 1-column matmul per head.

Per core (4 batches, T=2048 tokens):
  QK phase: Q/K projections for ALL batches weight-stationary (fp8-DR);
    per-head sq-norms via selector DR matmuls; rq folded into Q^T
    (bcast matmul + mul), rk*64 transposed to per-k-token columns (used as
    the per-partition scale of the e8 cast).
  Per batch: V proj (fp8-DR, x^T blocks as weights); scores^T = K^T Q-hat
    per head-half (bf16); e8 = scores * rk64 (fp8, centered, no +1);
    AV + Vsum via fp8-DR sharing the loaded V weights; attb8 = (pav + pvs)
    * const; O proj fp8-DR; residual + mean-only LN1 (the rstd scaling
    commutes through relu/FFN into LN2); h1 -> h1T8 fp8 via PE transposes.
  FFN1 weight-stationary fp8-DR over all 2048 tokens; relu evict to fp8;
  FFN2 fp8-DR (activations as weights); residual + LN2 (Square-accum
  variance, rstd = exp(-0.5 ln(var+eps))).
"""

import sys

if "/opt/trn_rl_repo" not in sys.path:
    sys.path.insert(0, "/opt/trn_rl_repo")

import ml_dtypes
import numpy as np

import concourse.bass as bass
import concourse.tile as tile
from concourse import mybir
from concourse.bass_utils import run_bass_kernel_spmd

F32 = mybir.dt.float32
BF16 = mybir.dt.bfloat16
F8 = mybir.dt.float8e4
NPBF16 = ml_dtypes.bfloat16
NPF8 = ml_dtypes.float8_e4m3fn
AF = mybir.ActivationFunctionType
OP = mybir.AluOpType
DRM = mybir.MatmulPerfMode.DoubleRow

# problem constants
B, S, D = 32, 512, 512
H, DK, DV, DFF = 8, 64, 64, 2048
TEMP = float(np.sqrt(DK))
LN_EPS = 1e-5
NCORES = 8
BPC = B // NCORES          # batches per core
T = BPC * S                # tokens per core
DC = D // 128              # d chunks
KCP = DC // 2              # d chunk pairs (DR)
FC = DFF // 128            # dff chunks
FCP = FC // 2              # dff chunk pairs
SB = S // 128              # token chunks per batch
P = 128

# fixed activation scale exponents (weights get dynamic exponents)
KH = 4     # h1 -> h1T8
KA = 7     # attn out -> attb8
KVB = 3    # v -> Vb8
KF = 4     # relu out -> ffa8


def ts(i, n):
    return slice(i * n, (i + 1) * n)


# walrus codegen caps on semaphore-wait commands per instruction (empirical);
# excess waits are moved onto chained same-engine NOPs ahead of the instruction.
_WAIT_CAPS = {}
_DEFAULT_WAIT_CAP = 1
_NOP_WAIT_CAP = 1


def _legalize_waits(nc):
    nop_id = [0]
    for f in nc.m.functions:
        for bb in f.blocks:
            insts = bb.instructions
            i = 0
            while i < len(insts):
                ins = insts[i]
                si = ins.sync_info
                cap = _WAIT_CAPS.get(type(ins).__name__, _DEFAULT_WAIT_CAP)
                if si is not None and si.on_wait and len(si.on_wait) > cap:
                    waits = list(si.on_wait)
                    keep = waits[-cap:] if cap > 0 else []
                    excess = waits[: len(waits) - cap]
                    new_nops = []
                    for j in range(0, len(excess), _NOP_WAIT_CAP):
                        chunk = excess[j: j + _NOP_WAIT_CAP]
                        nop = mybir.InstNoOp(
                            name=f"waitnop-{nop_id[0]}",
                            engine=ins.engine,
                            ins=[],
                            outs=[],
                            sync_info=mybir.SyncInfo(on_wait=chunk, on_update=[]),
                        )
                        nop_id[0] += 1
                        nc.register_instruction(nop)
                        new_nops.append(nop)
                    si.on_wait[:] = keep
                    insts[i:i] = new_nops
                    i += len(new_nops)
                i += 1


def _dedup_ldweights(nc):
    """Remove an InstLdweights when the PE array already holds the same
    weights (same AP + perf_mode + transpose-mode, no intervening transpose).
    LDWs carry no sem updates; waits migrate onto the next kept PE inst."""
    for f in nc.m.functions:
        for bb in f.blocks:
            insts = bb.instructions
            out = []
            last_w = None
            pend_waits = []
            for ins in insts:
                tn = type(ins).__name__
                if tn == "InstLdweights":
                    w = (str(ins.ins[0]), str(getattr(ins, "perf_mode", None)),
                         bool(getattr(ins, "is_transpose", False)))
                    if w == last_w:
                        si = ins.sync_info
                        if si is not None and si.on_wait:
                            pend_waits.extend(list(si.on_wait))
                        continue
                    last_w = w
                elif tn == "InstMatmult":
                    if getattr(ins, "is_transpose", False):
                        last_w = None
                if pend_waits and tn in ("InstLdweights", "InstMatmult"):
                    si = ins.sync_info
                    if si is None:
                        ins.sync_info = mybir.SyncInfo(on_wait=list(pend_waits),
                                                       on_update=[])
                    else:
                        si.on_wait.extend(pend_waits)
                    pend_waits = []
                out.append(ins)
            assert not pend_waits
            insts[:] = out


def build_program(apply_gb1, apply_gb2, apply_bf2, apply_bf1,
                  KQ, KK, KV, KO, K1, K2, MBF2):
    ln1_fast = (not apply_gb1) and (not apply_bf1)
    nc = bass.Bass("TRN2", target_bir_lowering=False, debug=False)

    # ---- DRAM I/O ----
    x_d = nc.dram_tensor("x", [T, D], F32, kind="ExternalInput")
    xT8_d = nc.dram_tensor("xT8", [P, KCP, 2, T], F8, kind="ExternalInput")
    wq8_d = nc.dram_tensor("wq8", [P, KCP, 2, D], F8, kind="ExternalInput")
    wk8_d = nc.dram_tensor("wk8", [P, KCP, 2, D], F8, kind="ExternalInput")
    wv8_d = nc.dram_tensor("wv8", [P, KCP, 2, D], F8, kind="ExternalInput")
    woA8_d = nc.dram_tensor("woA8", [DV, KCP, 2, D], F8, kind="ExternalInput")
    woB8_d = nc.dram_tensor("woB8", [DV, KCP, 2, D], F8, kind="ExternalInput")
    wf18_d = nc.dram_tensor("wf18", [P, KCP, 2, DFF], F8, kind="ExternalInput")
    wf28_d = nc.dram_tensor("wf28", [P, FCP, 2, D], F8, kind="ExternalInput")
    ss8_d = nc.dram_tensor("ss8", [P, KCP, 2, 32], F8, kind="ExternalInput")
    sbc_d = nc.dram_tensor("selbc", [H, DC, P], BF16, kind="ExternalInput")
    o2c_d = nc.dram_tensor("o2c8", [P, 2, 1], F8, kind="ExternalInput")
    idf_d = nc.dram_tensor("identf", [P, P], F32, kind="ExternalInput")
    idb_d = nc.dram_tensor("identb", [P, P], BF16, kind="ExternalInput")
    bf1_d = nc.dram_tensor("bf1s", [P, FC], F32, kind="ExternalInput")
    bf2_d = nc.dram_tensor("bf2b", [P, D], F32, kind="ExternalInput")
    g1_d = nc.dram_tensor("g1b", [P, D], F32, kind="ExternalInput")
    b1_d = nc.dram_tensor("b1b", [P, D], F32, kind="ExternalInput")
    g2_d = nc.dram_tensor("g2b", [P, D], F32, kind="ExternalInput")
    b2_d = nc.dram_tensor("b2b", [P, D], F32, kind="ExternalInput")
    out_d = nc.dram_tensor("out", [T, D], F32, kind="ExternalOutput")

    with tile.TileContext(nc) as tc:
        with tc.tile_pool(name="consts", bufs=1) as consts, \
             tc.tile_pool(name="big", bufs=1) as big:

            # ---- constants / weights (QK-phase tensors first) ----
            xT8 = big.tile([P, KCP, 2, T], F8)
            nc.sync.dma_start(xT8[:], xT8_d[:])
            wq8 = consts.tile([P, KCP, 2, D], F8)
            nc.sync.dma_start(wq8[:], wq8_d[:])
            wk8 = consts.tile([P, KCP, 2, D], F8)
            nc.sync.dma_start(wk8[:], wk8_d[:])
            ss8 = consts.tile([P, KCP, 2, 32], F8)
            nc.sync.dma_start(ss8[:], ss8_d[:])
            selbc = consts.tile([H, DC, P], BF16)
            nc.sync.dma_start(selbc[:], sbc_d[:])
            idb = consts.tile([P, P], BF16)
            nc.sync.dma_start(idb[:], idb_d[:])
            wv8 = consts.tile([P, KCP, 2, D], F8)
            nc.sync.dma_start(wv8[:], wv8_d[:])
            woA8 = consts.tile([DV, KCP, 2, D], F8)
            nc.sync.dma_start(woA8[:], woA8_d[:])
            woB8 = consts.tile([DV, KCP, 2, D], F8)
            nc.sync.dma_start(woB8[:], woB8_d[:])
            o2c = consts.tile([P, 2, 1], F8)
            nc.sync.dma_start(o2c[:], o2c_d[:])
            idf = consts.tile([P, P], F32)
            nc.sync.dma_start(idf[:], idf_d[:])
            wf18 = consts.tile([P, KCP, 2, DFF], F8)
            nc.sync.dma_start(wf18[:], wf18_d[:])
            wf28 = consts.tile([P, FCP, 2, D], F8)
            nc.sync.dma_start(wf28[:], wf28_d[:])
            ln64c = consts.tile([P, 1], F32)
            nc.vector.memset(ln64c[:], float(np.log(64.0)))
            eps128 = consts.tile([P, 1], F32)
            nc.vector.memset(eps128[:], LN_EPS)
            g1b = b1b = g2b = b2b = bf1s = bf2b = None
            if apply_gb1:
                g1b = consts.tile([P, D], F32)
                nc.sync.dma_start(g1b[:], g1_d[:])
                b1b = consts.tile([P, D], F32)
                nc.sync.dma_start(b1b[:], b1_d[:])
            if apply_gb2:
                g2b = consts.tile([P, D], F32)
                nc.sync.dma_start(g2b[:], g2_d[:])
                b2b = consts.tile([P, D], F32)
                nc.sync.dma_start(b2b[:], b2_d[:])
            if apply_bf1:
                bf1s = consts.tile([P, FC], F32)
                nc.sync.dma_start(bf1s[:], bf1_d[:])
            if apply_bf2:
                bf2b = consts.tile([P, D], F32)
                nc.sync.dma_start(bf2b[:], bf2_d[:])

            QT = big.tile([P, DC, T], BF16)     # raw Q^T (pre-fold)
            Q8T = big.tile([P, DC, 2, T], F8)   # [.,c,0,.]=Q-hat, [.,c,1,.]=0
            KT8 = big.tile([P, DC, 2, T], F8)   # [.,c,0,.]=K^T,   [.,c,1,.]=0
            nc.gpsimd.memset(Q8T[:, :, 1, :], 0.0)
            nc.gpsimd.memset(KT8[:, :, 1, :], 0.0)
            rkT = big.tile([P, BPC, H, SB], F32)
            h1tok = big.tile([P, T // P, D], F32)
            h1T8 = big.tile([P, DC, T], F8)     # dim1 = d-chunk
            ffa8 = big.tile([P, FCP, 2, T], F8)

            with tc.tile_pool(name="att", bufs=2) as ab, \
                 tc.tile_pool(name="ep", bufs=4) as ep, \
                 tc.tile_pool(name="bt", bufs=3) as bt, \
                 tc.tile_pool(name="psM", bufs=4, space="PSUM") as psM, \
                 tc.tile_pool(name="psS", bufs=2, space="PSUM") as psS, \
                 tc.tile_pool(name="psX", bufs=2, space="PSUM") as psX:

                # ---- QK phase: projections for all batches ----
                for w8, isq in ((wq8, True), (wk8, False)):
                    pfx = "q" if isq else "k"
                    sq8s = []
                    for b in range(BPC):
                        sq8s.append(bt.tile([P, KCP, 2, S], F8, tag="sq8",
                                            bufs=4, name=f"sq8_{pfx}{b}"))
                    for c in range(DC):
                        pps = []
                        for b in range(BPC):
                            pp = psM.tile([P, S], F32, tag="m", name=f"pp{b}")
                            pps.append(pp)
                        for kcp in range(KCP):
                            for b in range(BPC):
                                nc.tensor.matmul(
                                    pps[b][:], w8[:, kcp, :, ts(c, P)],
                                    xT8[:, kcp, :, ts(b, S)],
                                    start=(kcp == 0), stop=(kcp == KCP - 1),
                                    perf_mode=DRM)
                        for b in range(BPC):
                            xsl = (QT[:, c, ts(b, S)] if isq
                                   else KT8[:, c, 0, ts(b, S)])
                            nc.scalar.activation(
                                xsl, pps[b][:], AF.Copy,
                                scale=2.0 ** (-KQ if isq else -KK))
                            nc.vector.tensor_mul(sq8s[b][:, c // 2, c % 2, :],
                                                 xsl, xsl)
                    for b in range(BPC):
                        ps8 = psX.tile([32, S], F32, tag="x", name="ps8")
                        for cp in range(KCP):
                            nc.tensor.matmul(ps8[:], ss8[:, cp, :, :],
                                             sq8s[b][:, cp, :, :],
                                             start=(cp == 0),
                                             stop=(cp == KCP - 1),
                                             perf_mode=DRM)
                        t8 = bt.tile([H, S], F32, tag="t8", name="t8")
                        r8 = bt.tile([H, S], BF16, tag="r8", name="r8")
                        if isq:
                            # rq = exp(-.5*ln(ssq*temp^2)); fold into QT
                            nc.scalar.activation(t8[:], ps8[0:H, :], AF.Ln,
                                                 scale=TEMP * TEMP)
                            nc.scalar.activation(r8[:], t8[:], AF.Exp,
                                                 scale=-0.5)
                            for c in range(DC):
                                pb = psS.tile([P, S], F32, tag="s", name="pb")
                                nc.tensor.matmul(pb[:], selbc[:, c, :], r8[:],
                                                 start=True, stop=True)
                                nc.vector.tensor_mul(Q8T[:, c, 0, ts(b, S)],
                                                      QT[:, c, ts(b, S)], pb[:])
                        else:
                            # rk64 = 64*exp(-.5*ln(ssq)); transpose to columns
                            nc.scalar.activation(t8[:], ps8[0:H, :], AF.Ln)
                            nc.scalar.activation(r8[:], t8[:], AF.Exp,
                                                 scale=-0.5,
                                                 bias=ln64c[0:H, 0:1])
                            for j in range(SB):
                                pst = psX.tile([P, H], BF16, tag="x",
                                               name="pst")
                                nc.tensor.transpose(pst[:], r8[:, ts(j, P)],
                                                    idb[0:H, 0:H])
                                nc.scalar.activation(rkT[:, b, :, j], pst[:],
                                                     AF.Copy)

                # ---- per-batch attention ----
                for b in range(BPC):
                    Vb8 = ab.tile([P, SB, D], F8, tag="Vb8")
                    attbA = ab.tile([DV, KCP, 2, S], F8, tag="attbA")
                    attbB = ab.tile([DV, KCP, 2, S], F8, tag="attbB")
                    xtb = ab.tile([P, SB, D], F32, tag="xtb", bufs=1)
                    nc.sync.dma_start(
                        xtb[:], x_d[ts(b, S), :].rearrange("(q p) d -> p q d",
                                                           p=P))
                    for q in range(SB):
                        pv = psM.tile([P, D], F32, tag="m", name="pv")
                        for kcp in range(KCP):
                            nc.tensor.matmul(
                                pv[:], xT8[:, kcp, :, ts(b * SB + q, P)],
                                wv8[:, kcp, :, :],
                                start=(kcp == 0), stop=(kcp == KCP - 1),
                                perf_mode=DRM)
                        nc.vector.tensor_scalar_mul(Vb8[:, q, :], pv[:],
                                                    2.0 ** (KVB - KV))
                    for c in range(DC):
                        for half in range(2):
                            h = 2 * c + half
                            r0 = DV * half
                            pav = psM.tile([DV, S], F32, tag="m", name="pav")
                            pvs = psX.tile([DV, 1], F32, tag="x", name="pvs")
                            for jp in range(2):
                                e8t = ep.tile([P, 2, S], F8, tag="e8",
                                              name=f"e8_{half}_{jp}")
                                for j2 in range(2):
                                    j = jp * 2 + j2
                                    pscr = psS.tile([P, S], F32, tag="s",
                                                    name="pscr")
                                    nc.tensor.matmul(
                                        pscr[:],
                                        KT8[r0:r0 + DV, c, :,
                                            b * S + j * P:b * S + (j + 1) * P],
                                        Q8T[r0:r0 + DV, c, :, ts(b, S)],
                                        start=True, stop=True,
                                        perf_mode=DRM)
                                    rk_ap = rkT[:, b, h, j:j + 1]
                                    if j2 == 0:
                                        nc.vector.tensor_scalar(
                                            e8t[:, 0, :], pscr[:], rk_ap,
                                            None, OP.mult)
                                    else:
                                        nc.scalar.activation(
                                            e8t[:, 1, :], pscr[:], AF.Copy,
                                            scale=rk_ap)
                                vpair = Vb8[:, 2 * jp:2 * jp + 2,
                                            h * DV:(h + 1) * DV]
                                nc.tensor.matmul(pav[:], vpair,
                                                 e8t[:, :, :],
                                                 start=(jp == 0),
                                                 stop=(jp == 1),
                                                 perf_mode=DRM)
                                nc.tensor.matmul(pvs[:], vpair,
                                                 o2c[:, :, :],
                                                 start=(jp == 0),
                                                 stop=(jp == 1),
                                                 perf_mode=DRM)
                            pvsb = bt.tile([DV, 1], F32, tag="pvsb",
                                           name="pvsb")
                            nc.scalar.activation(pvsb[:], pvs[:], AF.Copy)
                            # attb = (pav + Vsum) * 2^KA / (2^KVB * 64 * S)
                            ca = 2.0 ** KA / (2.0 ** KVB * 64.0 * S)
                            dsth = attbA if half == 0 else attbB
                            nc.vector.tensor_scalar(
                                dsth[:, c // 2, c % 2, :],
                                pav[:], pvsb[:, 0:1], ca, OP.add, OP.mult)

                    # O-projection + residual + LN1 + transpose to h1T8
                    for q in range(SB):
                        idx = b * SB + q
                        po = psM.tile([P, D], F32, tag="m", name="po")
                        for cp in range(KCP):
                            for hf, (at, wt) in enumerate(((attbA, woA8),
                                                           (attbB, woB8))):
                                nc.tensor.matmul(po[:], at[:, cp, :, ts(q, P)],
                                                 wt[:, cp, :, :],
                                                 start=(cp == 0 and hf == 0),
                                                 stop=(cp == KCP - 1 and hf == 1),
                                                 perf_mode=DRM)
                        oscale = 2.0 ** (-KA - KO)
                        h1 = h1tok[:, idx, :]
                        if ln1_fast:
                            r1 = bt.tile([P, D], F32, tag="r1", name="r1")
                            s1 = bt.tile([P, 1], F32, tag="s1", name="s1")
                            nc.vector.scalar_tensor_tensor(
                                r1[:], po[:], oscale, xtb[:, q, :],
                                op0=OP.mult, op1=OP.add, accum_out=s1[:])
                            nm = bt.tile([P, 1], F32, tag="nm", name="nm")
                            nc.vector.tensor_scalar_mul(nm[:], s1[:],
                                                        -1.0 / D)
                            nc.vector.tensor_scalar(h1, r1[:], nm[:, 0:1],
                                                    None, OP.add)
                        else:
                            r1 = bt.tile([P, D], F32, tag="r1", name="r1")
                            nc.vector.scalar_tensor_tensor(
                                r1[:], po[:], oscale, xtb[:, q, :],
                                op0=OP.mult, op1=OP.add)
                            bst = bt.tile([P, 6], F32, tag="bst", name="bst")
                            nc.vector.bn_stats(bst[:], r1[:])
                            mv = bt.tile([P, 2], F32, tag="mv", name="mv")
                            nc.vector.bn_aggr(mv[:], bst[:])
                            lv = bt.tile([P, 1], F32, tag="lv", name="lv")
                            nc.scalar.activation(lv[:], mv[:, 1:2], AF.Ln,
                                                 bias=eps128[:, 0:1])
                            rstd = bt.tile([P, 1], F32, tag="rstd",
                                           name="rstd")
                            nc.scalar.activation(rstd[:], lv[:], AF.Exp,
                                                 scale=-0.5)
                            nc.vector.tensor_scalar(h1, r1[:], mv[:, 0:1],
                                                    rstd[:, 0:1],
                                                    OP.subtract, OP.mult)
                            if apply_gb1:
                                nc.vector.tensor_mul(h1, h1, g1b[:])
                                nc.vector.tensor_add(h1, h1, b1b[:])
                        ptt = psX.tile([P, DC, P], F32, tag="x", name="ptt")
                        for c in range(DC):
                            nc.tensor.transpose(ptt[:, c, :],
                                                h1[:, ts(c, P)], idf[:])
                        nc.scalar.activation(h1T8[:, :, ts(idx, P)], ptt[:],
                                             AF.Copy, scale=2.0 ** KH)

            # ---- FFN1 + FFN2 + LN2 ----
            with tc.tile_pool(name="psF", bufs=4, space="PSUM") as psF, \
                 tc.tile_pool(name="ft", bufs=3) as ft:
                rscale = 2.0 ** (KF - K1 - KH)
                for f in range(FC):
                    pf2s = []
                    for pr in range(2):
                        pf2 = psF.tile([P, 2, S], F32, tag="f",
                                       name=f"pf2_{pr}")
                        pf2s.append(pf2)
                    for cp in range(KCP):
                        for tb in range(BPC):
                            nc.tensor.matmul(
                                pf2s[tb // 2][:, tb % 2, :],
                                wf18[:, cp, :, ts(f, P)],
                                h1T8[:, 2 * cp:2 * cp + 2, ts(tb, S)],
                                start=(cp == 0), stop=(cp == KCP - 1),
                                perf_mode=DRM)
                    rbias = bf1s[:, f:f + 1] if apply_bf1 else 0.0
                    for pr in range(2):
                        dst = ffa8[:, f // 2, f % 2, ts(pr, 2 * S)]
                        src = pf2s[pr][:].rearrange("p a b -> p (a b)")
                        if apply_bf1 or pr == 0:
                            nc.scalar.activation(dst, src, AF.Relu,
                                                 scale=rscale, bias=rbias)
                        else:
                            nc.vector.tensor_scalar(dst, src, rscale, 0.0,
                                                    OP.mult, OP.max)

                f2scale = 2.0 ** (-KF - K2)
                for tb in range(BPC):
                    r2s = []
                    sum4 = ft.tile([P, SB], F32, tag="sum4", name="sum4")
                    ssq4 = ft.tile([P, SB], F32, tag="ssq4", name="ssq4")
                    for q in range(SB):
                        tok = tb * SB + q
                        p2 = psF.tile([P, D], F32, tag="f", name="p2")
                        for fp in range(FCP):
                            nc.tensor.matmul(p2[:],
                                             ffa8[:, fp, :, ts(tok, P)],
                                             wf28[:, fp, :, :],
                                             start=(fp == 0),
                                             stop=(fp == FCP - 1),
                                             perf_mode=DRM)
                        r2 = ft.tile([P, D], F32, tag=f"r2_{q}",
                                     name=f"r2_{q}")
                        nc.vector.scalar_tensor_tensor(
                            r2[:], p2[:], f2scale, h1tok[:, tok, :],
                            op0=OP.mult, op1=OP.add,
                            accum_out=sum4[:, q:q + 1])
                        if apply_bf2:
                            nc.vector.tensor_add(r2[:], r2[:], bf2b[:])
                        junk = ft.tile([P, D], F32, tag="junk", name="junk")
                        nc.scalar.activation(junk[:], r2[:], AF.Square,
                                             accum_out=ssq4[:, q:q + 1])
                        r2s.append(r2)
                    mean4 = ft.tile([P, SB], F32, tag="mean4", name="mean4")
                    if apply_bf2:
                        # bf2 shifts every token mean by the constant MBF2
                        nc.vector.tensor_scalar(mean4[:], sum4[:], 1.0 / D,
                                               MBF2, OP.mult, OP.add)
                    else:
                        nc.vector.tensor_scalar(mean4[:], sum4[:], 1.0 / D,
                                               None, OP.mult)
                    m2t = ft.tile([P, SB], F32, tag="m2t", name="m2t")
                    nc.vector.tensor_mul(m2t[:], mean4[:], mean4[:])
                    var4 = ft.tile([P, SB], F32, tag="var4", name="var4")
                    nc.vector.scalar_tensor_tensor(var4[:], ssq4[:], 1.0 / D,
                                                   m2t[:], op0=OP.mult,
                                                   op1=OP.subtract)
                    lv4 = ft.tile([P, SB], F32, tag="lv4", name="lv4")
                    nc.scalar.activation(lv4[:], var4[:], AF.Ln,
                                         bias=eps128[:, 0:1])
                    rstd4 = ft.tile([P, SB], F32, tag="rstd4", name="rstd4")
                    nc.scalar.activation(rstd4[:], lv4[:], AF.Exp, scale=-0.5)
                    for q in range(SB):
                        y = ft.tile([P, D], F32, tag="y", name="y")
                        nc.vector.tensor_scalar(y[:], r2s[q][:],
                                                mean4[:, q:q + 1],
                                                rstd4[:, q:q + 1],
                                                OP.subtract, OP.mult)
                        if apply_gb2:
                            nc.vector.tensor_mul(y[:], y[:], g2b[:])
                            nc.vector.tensor_add(y[:], y[:], b2b[:])
                        nc.sync.dma_start(out_d[ts(tb * SB + q, P), :], y[:])

    _dedup_ldweights(nc)
    _legalize_waits(nc)
    return nc


_CACHED_NC = {}


def _get_nc(key):
    if key not in _CACHED_NC:
        _CACHED_NC[key] = build_program(*key)
    return _CACHED_NC[key]


def _kexp(w):
    m = float(np.abs(w).max())
    if m == 0.0:
        return 0
    return int(np.clip(np.floor(np.log2(240.0 / m)), -8, 20))


def _make_consts():
    pp4 = np.arange(P)[:, None, None, None]
    cps = np.arange(KCP)[None, :, None, None]
    i2 = np.arange(2)[None, None, :, None]
    hh4 = np.arange(32)[None, None, None, :]
    ss8 = (hh4 == 2 * (2 * cps + i2) + pp4 // 64)
    hh = np.arange(H)
    pp = np.arange(P)
    cc = np.arange(DC)
    selbc = (hh[:, None, None] == 2 * cc[None, :, None] + pp[None, None, :] // 64)
    return {
        "ss8": ss8.astype(NPF8),
        "selbc": selbc.astype(NPBF16),
        "o2c8": np.full((P, 2, 1), 64.0, dtype=NPF8),
        "identf": np.eye(P, dtype=np.float32),
        "identb": np.eye(P, dtype=NPBF16),
    }


def _pack_wo_half(w, kexp, half):
    """[D, N] -> [DV, KCP, 2, N] fp8 with row (cp*2+i)*128 + half*64 + dv."""
    ws = (np.asarray(w, np.float32) * (2.0 ** kexp)).reshape(KCP, 2, 2, DV, -1)
    return np.ascontiguousarray(ws[:, :, half].transpose(2, 0, 1, 3)).astype(NPF8)


def _pack_w(w, kexp, npairs):
    """[K, N] -> [P, npairs, 2, N] fp8 with row (pair*2+i)*128 + p."""
    K, N = w.shape
    assert K == npairs * 2 * P
    ws = (np.asarray(w, np.float32) * (2.0 ** kexp)).reshape(npairs, 2, P, N)
    return np.ascontiguousarray(ws.transpose(2, 0, 1, 3)).astype(NPF8)


def make_in_maps(x, w_q, w_k, w_v, w_o, w_ff1, b_ff1, w_ff2, b_ff2,
                 g1, b1, g2, b2):
    f = np.float32
    kq, kk, kv, ko = _kexp(w_q), _kexp(w_k), _kexp(w_v), _kexp(w_o)
    k1, k2 = _kexp(w_ff1), _kexp(w_ff2)
    mbf2 = float(np.asarray(b_ff2, f).sum() / D)
    make_in_maps.scales = (kq, kk, kv, ko, k1, k2, mbf2)
    shared = {
        "wq8": _pack_w(np.asarray(w_q, f), kq, KCP),
        "wk8": _pack_w(np.asarray(w_k, f), kk, KCP),
        "wv8": _pack_w(np.asarray(w_v, f), kv, KCP),
        "woA8": _pack_wo_half(np.asarray(w_o, f), ko, 0),
        "woB8": _pack_wo_half(np.asarray(w_o, f), ko, 1),
        "wf18": _pack_w(np.asarray(w_ff1, f), k1, KCP),
        "wf28": _pack_w(np.asarray(w_ff2, f), k2, FCP),
        "bf1s": np.ascontiguousarray(
            np.asarray(b_ff1, f).reshape(FC, P).T * (2.0 ** KF)),
        "bf2b": np.broadcast_to(np.asarray(b_ff2, f), (P, D)).copy(),
        "g1b": np.broadcast_to(np.asarray(g1, f), (P, D)).copy(),
        "b1b": np.broadcast_to(np.asarray(b1, f), (P, D)).copy(),
        "g2b": np.broadcast_to(np.asarray(g2, f), (P, D)).copy(),
        "b2b": np.broadcast_to(np.asarray(b2, f), (P, D)).copy(),
        **_make_consts(),
    }
    x = np.ascontiguousarray(np.asarray(x, f))
    maps = []
    for c in range(NCORES):
        xc = x[ts(c, BPC)].reshape(T, D)
        xT = np.ascontiguousarray(xc.T)  # [D, T]
        xT8 = np.ascontiguousarray(
            xT.reshape(KCP, 2, P, T).transpose(2, 0, 1, 3)).astype(NPF8)
        maps.append({"x": xc, "xT8": xT8, **shared})
    return maps


def _flags_for(inputs):
    f = np.float32
    gb1 = (np.array_equal(np.asarray(inputs["g1"], f), np.ones(D, f))
           and np.array_equal(np.asarray(inputs["b1"], f), np.zeros(D, f)))
    gb2 = (np.array_equal(np.asarray(inputs["g2"], f), np.ones(D, f))
           and np.array_equal(np.asarray(inputs["b2"], f), np.zeros(D, f)))
    bf2 = bool(np.any(np.asarray(inputs["b_ff2"], f)))
    bf1 = bool(np.any(np.asarray(inputs["b_ff1"], f)))
    return (not gb1, not gb2, bf2, bf1)


def run(in_maps, flags=(False, False, False, False), **kw):
    scales = getattr(make_in_maps, "scales", None)
    assert scales is not None, "call make_in_maps first"
    nc = _get_nc(tuple(flags) + tuple(scales))
    return run_bass_kernel_spmd(nc, in_maps, core_ids=list(range(NCORES)), **kw)


def kernel(**inputs):
    flags = _flags_for(inputs)
    res = run(make_in_maps(**inputs), flags=flags)
    out = np.concatenate([r["out"].reshape(BPC, S, D) for r in res.results],
                         axis=0)
    return out.astype(np.float32)
